# revision 1
# baseline (speedup 1.0000x reference)
"""Additive (Bahdanau) attention kernel for Trainium2, 8 NeuronCores.

Reference computation (B=4, L=1024, D=512, U=64):
    k = x @ Wx                                   [B, L, U]
    q = x @ Wt                                   [B, L, U]
    h = tanh(q[:,i,None,:] + k[:,None,j,:] + bt) [B, L, L, U]
    e = exp(h . Wa + ba)                         [B, L, L]
    a = e / (sum_j e + 1e-7)
    v = a @ x                                    [B, L, D]

Sharding: core c handles batch b=c//2, query half h=c%2 (512 queries), all
1024 keys of that batch.

Per-core algorithm ("key-quad" formulation, scores computed transposed):
  - kT[u, j] (u on partitions) via PE; qT[u, i] likewise.
  - qq_h [128, NQ] = qT half-h (32 u's) replicated 4x along partitions.
  - kb[hh][q]: per-partition bias columns; col (gg, p) holds keys
    (128g + 32m + p), m=0..3 of u-half hh (+bt), for the 2 key-blocks of
    key-quarter q.
  - For each (g, p, hh): ARG = qq_hh + kb col   (DVE tensor_scalar, fp32)
    tanh on ACT in big batches -> TT (f32r)
    one full [128,128]-stationary matmul per (g, p, hh) accumulates rows
    (p, p+32, p+64, p+96) of the [128, NQ] transposed score block g in PSUM
    (stationary = host-built shifted block-diagonal of Wa halves, "dpad").
  - exp (+ba) from PSUM -> eT[j, i] in SBUF (f32r).
  - v_unscaled[i, d] = sum_j eT[j,i] x[j,d] via PE (lhsT = eT tiles);
    den[i] = sum_j eT[j,i] via PE with ones rhs.
  - v = v_unscaled * 1/(den + eps)  (DVE), DMA out.

Input DMAs are split across the three DMA paths (sync/scalar HWDGE rings +
gpsimd SWDGE) so the first tanh starts ~23us in: the scalar ring carries only
small tensors (its DMA instructions occupy the ACT queue), gpsimd carries only
data consumed well after its ~5us semaphore lag, and the x[j, d] AV tiles ride
gpsimd late, when HBM is free. Measured ~276-283 us over 13 runs, rel err
~2.2e-4 (fp32r matmul envelope).
"""

import numpy as np
import concourse.bass as bass
import concourse.mybir as mybir
import concourse.tile as tile
from concourse import bacc
from concourse.bass_utils import run_bass_kernel_spmd

F32 = mybir.dt.float32
F32R = mybir.dt.float32r
Act = mybir.ActivationFunctionType

B, L, D, U = 4, 1024, 512, 64
NCORES = 8
NQ = L // 2  # queries per core
EPS = 1e-7
B_TANH = 16  # max tanh tiles per ACT batch
NG = L // 128  # key blocks (8)
NQT = 4  # key quarters (256 key-cols each)
NI = NQ // 128  # query blocks (4)
DC = D // 128  # contraction chunks for projections (4)

_cached = {}


def _build():
    if "nc" in _cached:
        return _cached["nc"]
    nc = bacc.Bacc("TRN2", target_bir_lowering=False, debug=False, num_devices=NCORES)

    xbt = nc.dram_tensor("xbt", [D, L], F32R, kind="ExternalInput").ap()
    xqt = nc.dram_tensor("xqt", [D, NQ], F32R, kind="ExternalInput").ap()
    wx = nc.dram_tensor("wx", [128, DC, U], F32R, kind="ExternalInput").ap()
    wts = nc.dram_tensor("wts", [128, 2, DC, 128], F32R, kind="ExternalInput").ap()
    dpad = nc.dram_tensor("dpad", [128, 2, 255], F32R, kind="ExternalInput").ap()
    selk = nc.dram_tensor("selk", [64, 2, 4, 128], F32R, kind="ExternalInput").ap()
    btq = nc.dram_tensor("btq", [128, 2], F32, kind="ExternalInput").ap()
    bac = nc.dram_tensor("bac", [128, 1], F32, kind="ExternalInput").ap()
    onesv = nc.dram_tensor("onesv", [128, 8], F32R, kind="ExternalInput").ap()
    xb = nc.dram_tensor("xb", [L, D], F32R, kind="ExternalInput").ap()
    vout = nc.dram_tensor("v_out", [NQ, D], F32, kind="ExternalOutput").ap()

    from contextlib import ExitStack

    with tile.TileContext(nc) as tc, ExitStack() as ctx:
        const = ctx.enter_context(tc.tile_pool(name="const", bufs=1))
        xb_sb = [
            const.tile([128, D], F32R, tag=f"xbg{g}", name=f"xbg{g}")
            for g in range(NG)
        ]
        dpad_sb = const.tile([128, 2, 255], F32R, tag="dpad")
        selk_sb = const.tile([64, 2, 4, 128], F32R, tag="selk")
        btq_sb = const.tile([128, 2], F32, tag="btq")
        wx_sb = const.tile([128, DC, U], F32R, tag="wx")
        bac_sb = const.tile([128, 1], F32, tag="bac")
        qq_sb = const.tile([128, 2, NQ], F32, tag="qq")
        et_sb = const.tile([128, NG, NQ], F32R, tag="et")
        ones_sb = const.tile([128, 8], F32R, tag="ones")
        rcol_sb = const.tile([128, NI], F32, tag="rcol")
        tmp_sb = const.tile([128, NI], F32, tag="tmp")
        # per-(c, quarter) xbt pieces and per-quarter kT/kb tiles keep the
        # dependency graph fine-grained so consumers start per quarter
        xbt_sb = [
            [
                const.tile([128, 256], F32R, tag=f"xbt{c}_{q}", name=f"xbt{c}_{q}")
                for q in range(NQT)
            ]
            for c in range(DC)
        ]
        kt_sb = [
            const.tile([64, 256], F32R, tag=f"kt{q}", name=f"kt{q}")
            for q in range(NQT)
        ]
        kb_sb = [
            [
                const.tile([128, 64], F32, tag=f"kb{hh}_{q}", name=f"kb{hh}_{q}")
                for q in range(NQT)
            ]
            for hh in range(2)
        ]

        # ---------------- input DMAs ----------------
        # Two HWDGE rings (sync=qSP, scalar=qAct) carry the latency-critical
        # stream, quarter-interleaved so each key-quarter lands early and
        # whole; gpsimd SWDGE carries late-needed small tensors.
        xbt_r = xbt.rearrange("(c p) j -> p c j", p=128)
        xb_r = xb.rearrange("(g p) d -> p g d", p=128)
        xqt_r = xqt.rearrange("(c p) i -> p c i", p=128)

        with (
            tc.tile_pool(name="setup_sb", bufs=1) as ssb,
            tc.tile_pool(name="setup_ps", bufs=1, space="PSUM") as sps,
        ):
            xqt_sb = [
                ssb.tile([128, NQ], F32R, tag=f"xqt{c}", name=f"xqt{c}")
                for c in range(DC)
            ]
            wts_sb = ssb.tile([128, 2, DC, 128], F32R, tag="wts")

            # scalar (qAct) ring: ONLY small tensors — the DMA instructions
            # sit on the ACT engine queue, and ACT must stay free for tanh
            nc.scalar.dma_start(out=wts_sb[:], in_=wts[:])
            nc.scalar.dma_start(out=wx_sb[:], in_=wx[:])
            nc.scalar.dma_start(out=selk_sb[:], in_=selk[:])
            nc.scalar.dma_start(out=btq_sb[:], in_=btq[:])
            nc.scalar.dma_start(out=bac_sb[:], in_=bac[:])
            # sync (qSP) + gpsimd (SWDGE) rings: the bulk, quarter-interleaved
            # (SWDGE descriptor ring holds ~1024 descs -> keep its load small)
            nc.sync.dma_start(out=xqt_sb[0][:], in_=xqt_r[:, 0, :])
            nc.sync.dma_start(out=xqt_sb[1][:], in_=xqt_r[:, 1, :])
            nc.gpsimd.dma_start(out=xqt_sb[2][:], in_=xqt_r[:, 2, :])
            nc.gpsimd.dma_start(out=xqt_sb[3][:], in_=xqt_r[:, 3, :])
            for q in range(NQT):
                js = slice(q * 256, (q + 1) * 256)
                nc.sync.dma_start(out=xbt_sb[0][q][:], in_=xbt_r[:, 0, js])
                nc.sync.dma_start(out=xbt_sb[1][q][:], in_=xbt_r[:, 1, js])
                nc.sync.dma_start(out=xbt_sb[2][q][:], in_=xbt_r[:, 2, js])
                nc.gpsimd.dma_start(out=xbt_sb[3][q][:], in_=xbt_r[:, 3, js])
                if q == 0:
                    nc.gpsimd.dma_start(out=dpad_sb[:], in_=dpad[:])
                elif q == 1:
                    nc.gpsimd.dma_start(out=ones_sb[:], in_=onesv[:])
            # x[j, d] AV tiles ride the gpsimd ring late: HBM is free once
            # the latency-critical stream has landed
            for g in range(NG):
                nc.gpsimd.dma_start(out=xb_sb[g][:], in_=xb_r[:, g, :])

            # ---------------- projections ----------------
            # qq_h [128, NQ] = (Wt S_h)^T @ xq^T directly (host replicates
            # Wt half-h columns 4x) — gates every ARG add, so fewest hops
            for hh in range(2):
                qq_ps = sps.tile([128, NQ], F32, tag="qq_ps", bufs=2, name="qq_ps")
                for c in range(DC):
                    nc.tensor.matmul(
                        qq_ps[:],
                        wts_sb[:, hh, c, :],
                        xqt_sb[c][:],
                        start=(c == 0),
                        stop=(c == DC - 1),
                    )
                nc.vector.tensor_copy(qq_sb[:, hh, :], qq_ps[:])


        # ---------------- main loop ----------------
        arg_pool = ctx.enter_context(tc.tile_pool(name="arg", bufs=2))
        tt_pool = ctx.enter_context(tc.tile_pool(name="tt", bufs=2))
        sc_pool = ctx.enter_context(tc.tile_pool(name="sc", bufs=3, space="PSUM"))
        v_pool = ctx.enter_context(tc.tile_pool(name="vps", bufs=1, space="PSUM"))
        vo_pool = ctx.enter_context(tc.tile_pool(name="vo", bufs=2))

        v_tiles = [
            v_pool.tile([128, D], F32, tag=f"v{i}", name=f"v{i}") for i in range(NI)
        ]
        den_ps = v_pool.tile([128, NI, 8], F32, tag="den")

        def emit_quarter(q):
            # kT[u, 256q:256(q+1)] = Wx^T @ x^T[:, quarter] -> kb biases
            kt_ps = sc_pool.tile([64, 256], F32, tag="sc", name="kt_ps")
            for c in range(DC):
                nc.tensor.matmul(
                    kt_ps[:],
                    wx_sb[:, c, :],
                    xbt_sb[c][q][:],
                    start=(c == 0),
                    stop=(c == DC - 1),
                )
            nc.vector.tensor_copy(kt_sb[q][:], kt_ps[:])
            kt_view = kt_sb[q][:, :].rearrange("u (gg m p) -> u gg m p", m=4, p=32)
            for hh in range(2):
                kb_ps = sc_pool.tile([128, 64], F32, tag="sc", name="kb_ps")
                for m in range(4):
                    nc.tensor.matmul(
                        kb_ps[:],
                        selk_sb[:, hh, m, :],
                        kt_view[:, :, m, :],
                        start=(m == 0),
                        stop=(m == 3),
                    )
                nc.vector.tensor_scalar_add(
                    kb_sb[hh][q][:], kb_ps[:], btq_sb[:, hh : hh + 1]
                )

        seq = [(g, p, hh) for g in range(NG) for p in range(32) for hh in range(2)]
        nseq = len(seq)  # 512
        ramp_up = [1, 1, 2, 4, 4, 8, 12]
        ramp_down = [8, 4, 2, 1, 1]
        mid = nseq - sum(ramp_up) - sum(ramp_down)
        batch_sizes = list(ramp_up)
        while mid > 0:
            bsz = min(B_TANH, mid)
            batch_sizes.append(bsz)
            mid -= bsz
        batch_sizes += ramp_down
        assert sum(batch_sizes) == nseq
        bounds = []
        acc = 0
        for bsz in batch_sizes:
            bounds.append(acc)
            acc += bsz

        sc_tiles = {}
        arg_t = None
        batch = []  # (slot, g, p, hh) pending matmuls for current ACT batch

        def flush_batch():
            nonlocal batch
            if not batch:
                return
            ns = len(batch)
            tt_t = tt_pool.tile([128, B_TANH, NQ], F32R, tag="tt", name="tt_t")
            nc.scalar.activation(tt_t[:, :ns, :], arg_t[:, :ns, :], Act.Tanh)
            for slot, g, p, hh in batch:
                t_idx = g * 64 + p * 2 + hh
                if t_idx % 64 == 0:
                    sc_tiles[g] = sc_pool.tile([128, NQ], F32, tag="sc", name="sc_t")
                st = sc_tiles[g]
                nc.tensor.matmul(
                    st[:],
                    dpad_sb[:, hh, 127 - p : 255 - p],
                    tt_t[:, slot, :],
                    start=(t_idx % 64 == 0),
                    stop=(t_idx % 64 == 63),
                )
                if t_idx % 64 == 63:
                    # block g complete: exp -> eT, then AV + den partials.
                    # The last block's exp is split by i-chunk so its AV
                    # matmuls start as soon as their chunk is exp'd.
                    if g == NG - 1:
                        for ib in range(NI):
                            isl = slice(ib * 128, (ib + 1) * 128)
                            nc.scalar.activation(
                                et_sb[:, g, isl], st[:, isl], Act.Exp,
                                bias=bac_sb[:],
                            )
                    else:
                        nc.scalar.activation(
                            et_sb[:, g, :], st[:], Act.Exp, bias=bac_sb[:]
                        )
                    for ib in range(NI):
                        nc.tensor.matmul(
                            den_ps[:, ib, :],
                            et_sb[:, g, ib * 128 : (ib + 1) * 128],
                            ones_sb[:],
                            start=(g == 0 and ib == 0),
                            stop=(g == NG - 1 and ib == NI - 1),
                        )
                        nc.tensor.matmul(
                            v_tiles[ib][:],
                            et_sb[:, g, ib * 128 : (ib + 1) * 128],
                            xb_sb[g][:],
                            start=(g == 0),
                            stop=(g == NG - 1),
                        )
            batch = []

        emit_quarter(0)
        quarter_at = {64 + 128 * (q - 1): q for q in range(1, NQT)}
        next_b = 0
        slot = 0
        for t, (g, p, hh) in enumerate(seq):
            if t in quarter_at:
                emit_quarter(quarter_at[t])
            if next_b < len(bounds) and t == bounds[next_b]:
                flush_batch()
                arg_t = arg_pool.tile([128, B_TANH, NQ], F32, tag="arg", name="arg_t")
                next_b += 1
                slot = 0
            cc = 32 * (g % 2) + p
            nc.vector.tensor_scalar_add(
                arg_t[:, slot, :],
                qq_sb[:, hh, :],
                kb_sb[hh][g // 2][:, cc : cc + 1],
            )
            batch.append((slot, g, p, hh))
            slot += 1
        flush_batch()

        # ---------------- normalize + out ----------------
        for ib in range(NI):
            nc.vector.tensor_scalar_add(
                tmp_sb[:, ib : ib + 1], den_ps[:, ib, 0:1], float(EPS)
            )
            nc.vector.reciprocal(rcol_sb[:, ib : ib + 1], tmp_sb[:, ib : ib + 1])
            v_sb = vo_pool.tile([128, D], F32, tag="vsb", name="v_sb")
            nc.vector.tensor_scalar_mul(
                v_sb[:], v_tiles[ib][:], rcol_sb[:, ib : ib + 1]
            )
            nc.sync.dma_start(out=vout[ib * 128 : (ib + 1) * 128, :], in_=v_sb[:])

    nc.compile()
    _cached["nc"] = nc
    return nc


def _host_prep(x, Wx, Wt, bt, Wa, ba):
    x = np.ascontiguousarray(x, dtype=np.float32)
    Wx = np.ascontiguousarray(Wx, dtype=np.float32)
    Wt = np.ascontiguousarray(Wt, dtype=np.float32)
    bt = np.asarray(bt, dtype=np.float32).reshape(U)
    Wa = np.asarray(Wa, dtype=np.float32).reshape(U)
    ba = np.asarray(ba, dtype=np.float32).reshape(1)

    # dpad[:, h]: [128, 255]; col (127+32m) rows 32m..32m+31 hold Wa[32h:32h+32]
    dpad = np.zeros((128, 2, 255), dtype=np.float32)
    for h in range(2):
        for m in range(4):
            dpad[32 * m : 32 * m + 32, h, 127 + 32 * m] = Wa[32 * h : 32 * h + 32]

    # selk[:, h, m]: [64, 128]; only col block m populated
    selk = np.zeros((64, 2, 4, 128), dtype=np.float32)
    for h in range(2):
        for m in range(4):
            for up in range(32):
                selk[32 * h + up, h, m, 32 * m + up] = 1.0
    # wts[p, h, c, 32m+u'] = Wt[128c+p, 32h+u']  (Wt half-h cols replicated 4x)
    wts = np.empty((128, 2, DC, 128), dtype=np.float32)
    for h in range(2):
        half = Wt[:, 32 * h : 32 * h + 32]  # [D, 32]
        rep = np.tile(half, (1, 4)).reshape(DC, 128, 128)
        wts[:, h, :, :] = rep.transpose(1, 0, 2)

    btq = np.zeros((128, 2), dtype=np.float32)
    for h in range(2):
        btq[:, h] = np.tile(bt[32 * h : 32 * h + 32], 4)
    bac = np.full((128, 1), ba[0], dtype=np.float32)
    onesv = np.ones((128, 8), dtype=np.float32)

    Wxp = np.ascontiguousarray(Wx.reshape(DC, 128, U).transpose(1, 0, 2))
    shared = {
        "wx": Wxp, "wts": wts, "dpad": dpad, "selk": selk,
        "btq": btq, "bac": bac, "onesv": onesv,
    }

    in_maps = []
    for c in range(NCORES):
        b, h = c // 2, c % 2
        xbt_a = np.ascontiguousarray(x[b].T)
        xqt_a = np.ascontiguousarray(x[b, h * NQ : (h + 1) * NQ, :].T)
        m = dict(shared)
        m.update({"xbt": xbt_a, "xqt": xqt_a, "xb": x[b]})
        in_maps.append(m)
    return in_maps


def kernel(x, Wx, Wt, bt, Wa, ba):
    nc = _build()
    in_maps = _host_prep(x, Wx, Wt, bt, Wa, ba)
    res = run_bass_kernel_spmd(nc, in_maps, core_ids=list(range(NCORES)))
    out = np.empty((B, L, D), dtype=np.float32)
    for c in range(NCORES):
        b, h = c // 2, c % 2
        out[b, h * NQ : (h + 1) * NQ, :] = res.results[c]["v_out"]
    return out


if __name__ == "__main__":
    rng = np.random.default_rng(0)
    x = rng.standard_normal((B, L, D), dtype=np.float32)
    Wx = (rng.standard_normal((D, U), dtype=np.float32) * 0.06).astype(np.float32)
    Wt = (rng.standard_normal((D, U), dtype=np.float32) * 0.06).astype(np.float32)
    bt = np.zeros(U, dtype=np.float32)
    Wa = (rng.standard_normal((U, 1), dtype=np.float32) * 0.17).astype(np.float32)
    ba = np.zeros(1, dtype=np.float32)
    v = kernel(x=x, Wx=Wx, Wt=Wt, bt=bt, Wa=Wa, ba=ba)
    print("kernel ran, out shape", v.shape)

